# revision 20
# baseline (speedup 1.0000x reference)
"""RNN-T joint network kernel for 8 Trainium2 NeuronCores.

Reference computation:
    enc_proj = enc_out @ W_enc.T + b_enc          # [B,T,J]
    dec_proj = pred_out @ W_dec.T + b_dec         # [B,U,J]
    joint    = tanh(enc_proj[:,:,None,:] + dec_proj[:,None,:,:])
    out      = joint @ W_out.T + b_out            # [B,T,U,V]

Shapes (hardcoded): B=4, T=256, U=128, D=512, J=640, V=1024.

Sharding: data-parallel over the B*T = 1024 encoder rows; core k gets
batch b=k//2 and t-range [(k%2)*128, (k%2)*128+128).  Each core computes
its full [128, 128, 1024] output slab.

Algorithm (fp8 residual decomposition):
    tanh(x) = a*x + r(x), a = 0.6225 chosen to minimize Var[r]
    out = (E2[t,:] + D2[u,:]) + r(x)@Wo + b_out
  where E2 = a*e@Wo, D2 = a*d@Wo ride tiny f16 GEMMs, and the dominant
  GEMM contracts the small-variance residual r in fp8e4 (DoubleRow,
  2 fp8 MACs/cell/cycle).  Scales: r*64, Wo*4096 (TRN e4m3 max 240),
  PSUM carries 2^18, descale fused into the final combine DVE op.
  j-chunks 0-3 (512 of 640) run fp8-DR; chunk 4 runs direct-tanh f16
  at matched scale (same PE cost, less quantization error).
  Per-t E2[t,:] is added into PSUM via a K=1 float32r ones-matmul.
  Output is written fp16 (halves DMA); host upcasts to f32.
"""

import os
import numpy as np

B, T, U, D, J, V = 4, 256, 128, 512, 640, 1024
NCORES = 8
TC = (B * T) // NCORES          # 128 t-rows per core
JC = J // 128                   # 5 j-chunks
DC = D // 128                   # 4 d-chunks
NP = 2                          # fp8 DoubleRow chunk-pairs (j 0..511)
ALPHA = 0.6225                  # linear part of tanh split
SR = 64.0                       # residual fp8 scale
SW = 4096.0                     # W_out fp8 scale
S = SR * SW                     # PSUM scale (2^18)

_CACHE = {}


def _build_bass():
    import concourse.mybir as mybir
    import concourse.tile as tile
    import concourse.bacc as bacc

    f32 = mybir.dt.float32
    f32r = mybir.dt.float32r
    f16 = mybir.dt.float16
    f8 = mybir.dt.float8e4

    nc = bacc.Bacc("TRN2", debug=False)

    debug = bool(int(os.environ.get("TRNK_DEBUG", "0")))
    enc_d = nc.dram_tensor("enct", [D, TC], f16, kind="ExternalInput")
    pred_d = nc.dram_tensor("predt", [D, U], f16, kind="ExternalInput")
    wenc_d = nc.dram_tensor("wenct", [D, J], f16, kind="ExternalInput")
    wdec_d = nc.dram_tensor("wdect", [D, J], f16, kind="ExternalInput")
    w8p_d = nc.dram_tensor("w8p", [NP * 128, 2 * V], f8, kind="ExternalInput")
    w4s_d = nc.dram_tensor("w4s", [128, V], f16, kind="ExternalInput")
    wobh_d = nc.dram_tensor("wobh", [512, V], f16, kind="ExternalInput")
    abcomb_d = nc.dram_tensor("abcomb", [128, JC], f32, kind="ExternalInput")
    boutr_d = nc.dram_tensor("boutr", [128, V], f32, kind="ExternalInput")
    ones_d = nc.dram_tensor("onesr", [1, 128], f32, kind="ExternalInput")
    out_d = nc.dram_tensor("out", [TC, U, V], f16, kind="ExternalOutput")
    if debug:
        dbg_e2s = nc.dram_tensor("dbg_e2s", [128, V], f32, kind="ExternalOutput")
        dbg_d2b = nc.dram_tensor("dbg_d2b", [128, V], f32, kind="ExternalOutput")
        dbg_xp0 = nc.dram_tensor("dbg_xp0", [128, U], f16, kind="ExternalOutput")
        dbg_th0 = nc.dram_tensor("dbg_th0", [128, U], f16, kind="ExternalOutput")
        dbg_rq0 = nc.dram_tensor("dbg_rq0", [128, 2, U], f8, kind="ExternalOutput")
        dbg_t4 = nc.dram_tensor("dbg_t4", [128, U], f16, kind="ExternalOutput")
        dbg_ps0 = nc.dram_tensor("dbg_ps0", [128, V], f32, kind="ExternalOutput")
        dbg_e2r = nc.dram_tensor("dbg_e2r", [1, V], f32, kind="ExternalOutput")

    enc_ap, pred_ap = enc_d.ap(), pred_d.ap()
    wenc_ap, wdec_ap = wenc_d.ap(), wdec_d.ap()
    out_ap = out_d.ap()

    Tanh = mybir.ActivationFunctionType.Tanh
    Ident = mybir.ActivationFunctionType.Identity
    DR = mybir.MatmulPerfMode.DoubleRow

    with tile.TileContext(nc) as tc:
        with (
            tc.tile_pool(name="consts", bufs=1) as consts,
            tc.tile_pool(name="proj", bufs=1) as proj,
            tc.tile_pool(name="xpp", bufs=2 * JC) as xpp,
            tc.tile_pool(name="thp", bufs=2 * JC) as thp,
            tc.tile_pool(name="rqp", bufs=2 * NP) as rqp,
            tc.tile_pool(name="t4p", bufs=2) as t4p,
            tc.tile_pool(name="e2row", bufs=3) as e2rp,
            tc.tile_pool(name="osb", bufs=4) as osbp,
            tc.tile_pool(name="psB", bufs=4, space="PSUM") as psB,
        ):
            # ---- load inputs; projection operands first so PE can start ----
            enc_t, pred_t, wenc_t, wdec_t = [], [], [], []
            for dc in range(DC):
                sl = slice(dc * 128, (dc + 1) * 128)
                a = consts.tile([128, TC], f16, tag=f"enc{dc}")
                nc.sync.dma_start(a[:], enc_ap[sl, :])
                enc_t.append(a)
                p = consts.tile([128, U], f16, tag=f"pred{dc}")
                nc.sync.dma_start(p[:], pred_ap[sl, :])
                pred_t.append(p)
                we = consts.tile([128, J], f16, tag=f"wenc{dc}")
                nc.sync.dma_start(we[:], wenc_ap[sl, :])
                wenc_t.append(we)
                wd = consts.tile([128, J], f16, tag=f"wdec{dc}")
                nc.sync.dma_start(wd[:], wdec_ap[sl, :])
                wdec_t.append(wd)

            abcomb_t = consts.tile([128, JC], f32, tag="abcomb")
            nc.sync.dma_start(abcomb_t[:], abcomb_d.ap()[:])
            w8p_t = []
            for p8 in range(NP):
                w = consts.tile([128, 2, V], f8, tag=f"w8p{p8}")
                nc.sync.dma_start(
                    w[:], w8p_d.ap()[p8 * 128:(p8 + 1) * 128, :])
                w8p_t.append(w)
            w4s_t = consts.tile([128, V], f16, tag="w4s")
            nc.sync.dma_start(w4s_t[:], w4s_d.ap()[:])
            wobh_t = []
            for c in range(4):
                w = consts.tile([128, V], f16, tag=f"wobh{c}")
                nc.sync.dma_start(w[:], wobh_d.ap()[c * 128:(c + 1) * 128, :])
                wobh_t.append(w)
            boutr_t = consts.tile([128, V], f32, tag="boutr")
            nc.sync.dma_start(boutr_t[:], boutr_d.ap()[:])
            ones_t = consts.tile([1, 128], f32r, tag="onesr")
            nc.sync.dma_start(ones_t[:], ones_d.ap()[:].bitcast(f32r))

            # ---- projections: E[c][j,t] = a*e, D[c][j,u] = a*(d + bcomb) ----
            E_t, Ef_t, D_t = [], [], []
            for c in range(JC):
                jsl = slice(c * 128, (c + 1) * 128)
                pse = psB.tile([128, TC], f32, tag="ps")
                for dc in range(DC):
                    nc.tensor.matmul(pse[:], wenc_t[dc][:, jsl], enc_t[dc][:],
                                     start=(dc == 0), stop=(dc == DC - 1))
                e = proj.tile([128, TC], f16, tag=f"E{c}")
                nc.scalar.activation(e[:], pse[:], Ident, bias=0.0, scale=ALPHA)
                E_t.append(e)
                ef = proj.tile([128, TC], f32, tag=f"Ef{c}")
                nc.vector.tensor_copy(ef[:], e[:])
                Ef_t.append(ef)

                psd = psB.tile([128, U], f32, tag="ps")
                for dc in range(DC):
                    nc.tensor.matmul(psd[:], wdec_t[dc][:, jsl], pred_t[dc][:],
                                     start=(dc == 0), stop=(dc == DC - 1))
                d = proj.tile([128, U], f16, tag=f"D{c}")
                nc.scalar.activation(d[:], psd[:], Ident,
                                     bias=abcomb_t[:, c:c + 1], scale=ALPHA)
                D_t.append(d)

            # ---- mini-GEMMs: E2s[t,v] = S * E[:512].T @ Wo[:512],
            #                  D2b[u,v] = D[:512].T @ Wo[:512] + b_out ----
            pse2 = psB.tile([128, V], f32, tag="ps")
            for v in range(2):
                vsl = slice(v * 512, (v + 1) * 512)
                for c in range(4):
                    nc.tensor.matmul(pse2[:, vsl], E_t[c][:], wobh_t[c][:, vsl],
                                     start=(c == 0), stop=(c == 3))
            e2s = proj.tile([128, V], f32r, tag="e2s")
            nc.scalar.activation(e2s[:], pse2[:], Ident, bias=0.0, scale=S)

            psd2 = psB.tile([128, V], f32, tag="ps")
            for v in range(2):
                vsl = slice(v * 512, (v + 1) * 512)
                for c in range(4):
                    nc.tensor.matmul(psd2[:, vsl], D_t[c][:], wobh_t[c][:, vsl],
                                     start=(c == 0), stop=(c == 3))
            d2b = proj.tile([128, V], f32, tag="d2b")
            nc.vector.tensor_add(d2b[:], psd2[:], boutr_t[:])

            ones_r = ones_t[:]
            if debug:
                nc.sync.dma_start(dbg_e2s.ap()[:], e2s[:].bitcast(f32))
                nc.sync.dma_start(dbg_d2b.ap()[:], d2b[:])

            # ---- main loop over t ----
            prev = None  # (psum, t) pending combine
            for t in range(TC):
                # xp[j,u] = D[j,u] + E[j,t]; th = tanh(xp/a);
                # rq = (th - xp)*SR in fp8 (chunks 0-3), th*SR f16 (chunk 4)
                e2r = e2rp.tile([1, V], f32r, tag="e2r")
                nc.sync.dma_start(e2r[:], e2s[t:t + 1, :])
                xp_t, th_t = [], []
                for c in range(JC):
                    xp = xpp.tile([128, U], f16, tag=f"xp{c}")
                    nc.vector.tensor_scalar_add(xp[:], D_t[c][:],
                                                Ef_t[c][:, t:t + 1])
                    xp_t.append(xp)
                    th = thp.tile([128, U], f16, tag=f"th{c}")
                    nc.scalar.activation(th[:], xp[:], Tanh, bias=0.0,
                                         scale=1.0 / ALPHA)
                    th_t.append(th)
                rq_t = []
                for p8 in range(NP):
                    rq = rqp.tile([128, 2, U], f8, tag=f"rq{p8}")
                    for i in range(2):
                        c = p8 * 2 + i
                        nc.vector.ln_bwd_dx(rq[:, i, :], th_t[c][:],
                                            xp_t[c][:], 1.0, 0.0, scale=SR)
                    rq_t.append(rq)
                t4 = t4p.tile([128, U], f16, tag="t4")
                nc.vector.tensor_scalar_mul(t4[:], th_t[4][:], SR)
                if debug and t == 0:
                    nc.sync.dma_start(dbg_xp0.ap()[:], xp_t[0][:])
                    nc.sync.dma_start(dbg_th0.ap()[:], th_t[0][:])
                    nc.sync.dma_start(dbg_rq0.ap()[:], rq_t[0][:])
                    nc.sync.dma_start(dbg_t4.ap()[:], t4[:])
                    nc.sync.dma_start(dbg_e2r.ap()[:], e2r[:].bitcast(f32))

                # combine for previous t rides here so DVE doesn't stall PE
                if prev is not None:
                    ps_p, t_p = prev
                    osb = osbp.tile([128, V], f16, tag="osb")
                    nc.vector.affine_then_add(osb[:], ps_p[:], d2b[:],
                                              1.0 / S, 0.0)
                    nc.sync.dma_start(out_ap[t_p], osb[:])

                ps = psB.tile([128, V], f32, tag="ps")
                for v in range(2):
                    vsl = slice(v * 512, (v + 1) * 512)
                    for q in range(2):
                        qsl = slice((2 * v + q) * 256, (2 * v + q + 1) * 256)
                        wq = slice((2 * v + q) * 256, (2 * v + q + 1) * 256)
                        for p8 in range(NP):
                            # start=True clears the whole PSUM bank (512 f32),
                            # so only the first MM per v-half may set it.
                            nc.tensor.matmul(
                                ps[:, qsl], rq_t[p8][:, :, :],
                                w8p_t[p8][:, :, wq],
                                start=(p8 == 0 and q == 0), stop=False,
                                perf_mode=DR)
                    nc.tensor.matmul(ps[:, vsl], t4[:], w4s_t[:, vsl],
                                     start=False, stop=False)
                    nc.tensor.matmul(ps[:, vsl], ones_r[0:1, :],
                                     e2r[0:1, vsl],
                                     start=False, stop=True)
                if debug and t == 0:
                    pscp = osbp.tile([128, V], f32, tag="pscp")
                    nc.vector.tensor_copy(pscp[:], ps[:])
                    nc.sync.dma_start(dbg_ps0.ap()[:], pscp[:])
                prev = (ps, t)

            ps_p, t_p = prev
            osb = osbp.tile([128, V], f16, tag="osb")
            nc.vector.affine_then_add(osb[:], ps_p[:], d2b[:], 1.0 / S, 0.0)
            nc.sync.dma_start(out_ap[t_p], osb[:])

    nc.compile()
    return nc


def _host_prep(enc_out, pred_out, W_enc, b_enc, W_dec, b_dec, W_out, b_out):
    import concourse.mybir as mybir
    f8_np = np.dtype(mybir.dt.np(mybir.dt.float8e4))
    f16_np = np.float16

    wencT = np.ascontiguousarray(np.asarray(W_enc, np.float32).T).astype(f16_np)
    wdecT = np.ascontiguousarray(np.asarray(W_dec, np.float32).T).astype(f16_np)
    woT = np.ascontiguousarray(np.asarray(W_out, np.float32).T)  # [J, V]
    # fp8 DoubleRow pair layout: w8p[p8*128+p, i*V+v] = Wo[p8*256+i*128+p, v]*SW
    w8p = np.ascontiguousarray(
        (woT[:512] * SW).reshape(NP, 2, 128, V).transpose(0, 2, 1, 3)
        .reshape(NP * 128, 2 * V)).astype(f8_np)
    w4s = np.ascontiguousarray(woT[512:] * SW).astype(f16_np)
    wobh = np.ascontiguousarray(woT[:512]).astype(f16_np)
    abcomb = np.ascontiguousarray(
        (ALPHA * (np.asarray(b_enc, np.float32) + np.asarray(b_dec, np.float32)))
        .reshape(JC, 128).T)
    boutr = np.ascontiguousarray(
        np.broadcast_to(np.asarray(b_out, np.float32), (128, V)))
    onesr = np.ones((1, 128), np.float32)

    in_maps = []
    for k in range(NCORES):
        b, th = k // 2, (k % 2) * TC
        encT = np.ascontiguousarray(
            np.asarray(enc_out[b, th:th + TC], np.float32).T).astype(f16_np)
        predT = np.ascontiguousarray(
            np.asarray(pred_out[b], np.float32).T).astype(f16_np)
        in_maps.append({
            "enct": encT, "predt": predT, "wenct": wencT, "wdect": wdecT,
            "w8p": w8p, "w4s": w4s, "wobh": wobh, "abcomb": abcomb,
            "boutr": boutr, "onesr": onesr,
        })
    return in_maps


def kernel(enc_out, pred_out, W_enc, b_enc, W_dec, b_dec, W_out, b_out):
    from concourse import bass_utils

    if "nc" not in _CACHE:
        _CACHE["nc"] = _build_bass()
    nc = _CACHE["nc"]

    in_maps = _host_prep(enc_out, pred_out, W_enc, b_enc, W_dec, b_dec,
                         W_out, b_out)

    trace = bool(int(os.environ.get("TRNK_PROFILE", "0")))
    res = bass_utils.run_bass_kernel_spmd(
        nc, in_maps, core_ids=list(range(NCORES)), trace=trace)
    kernel.last_exec_ns = res.exec_time_ns

    full = np.empty((B, T, U, V), np.float32)
    for k in range(NCORES):
        b, th = k // 2, (k % 2) * TC
        full[b, th:th + TC] = res.results[k]["out"].astype(np.float32)
    return full


kernel.last_exec_ns = None
